# revision 2
# baseline (speedup 1.0000x reference)
"""MoE group-limited routing gate on 8 Trainium2 cores — v4.

Scheme (per core, 2048 tokens, D=7168, E=256):
  logits = x@w.T computed as  xh16@wh16  (fp16, full rate)
         + 2^-15 * (xl8@wh8 + xh8@wl8)   (fp8 E4M3, ONE DoubleRow pass:
                                          both cross terms ride the two
                                          DoubleRow planes)
  xh16 = fp16(x), xl8 = fp8(2^11 (x-xh16)), xh8 = fp8(xh16) cast on-device,
  wh8 = fp8(2^4 wh16), wl8 = fp8(2^15 (w-wh16)). fp16/fp8 PE products are
  exact (e10m23 accumulate) so the only error is input quantization:
  logit err ~2e-5 -> 9/131072 flipped topk indices (rel 6.1e-3).

Layout: w-stationary, k-major. PSUM accumulates [128 experts-half, W tok]
(main pair bufs=1 + res pair bufs=2 + 2 transpose banks = 8 banks).
Sub-phases of 512,512,512,256,256 tokens — the two half-width tail phases
shrink the post-last-matmul topk drain. Per k-chunk 4 MMs (2 fp16 + 2
DoubleRow) at 1 output column/cycle. Host prepacks every (sub-phase,
k-eighth) x block and every weight slice contiguously (3.5-7 KB runs);
input DMAs alternate across BOTH HWDGE rings (sync + scalar) so the
startup ramp gets combined bandwidth. Casts run on DVE, scores combine
on scalar+DVE, PE-transpose restores [t,e], then the DVE max8/max_index
group-limited top-k emits outputs on the gpsimd ring.
"""

import numpy as np
from contextlib import ExitStack

import concourse.bacc as bacc
import concourse.tile as tile
from concourse import mybir
from concourse.bass_utils import run_bass_kernel_spmd
from concourse.masks import make_identity

N_CORES = 8
T_FULL = 16384
D = 7168
E = 256
G = 8
EPG = E // G
TOPK = 8
TOPK_GROUPS = 4
ROUTE_SCALE = 2.5

P = 128
T = T_FULL // N_CORES        # 2048 tokens/core
KC = D // P                  # 56 k-chunks
SUBW = [512, 512, 512, 256, 256]     # sub-phase widths (sum = T)
SUBOFF = [0, 512, 1024, 1536, 1792]
NSUB = len(SUBW)
NQ = 8                       # k-eighths (x DMA granularity)
KQ = KC // NQ                # 7 k-chunks per eighth
WHS = 14                     # wh16 DMA slices
KWH = KC // WHS              # 4 k-chunks per wh16 slice
W8S = 8                      # w8 DMA slices
KW8 = KC // W8S              # 7 k-chunks per w8 slice
NEG = -1.0e30

S_XL, S_WH, S_WL = 11, 4, 15
DESCALE = 2.0 ** -(S_XL + S_WH)

f32 = mybir.dt.float32
f16 = mybir.dt.float16
f8 = mybir.dt.float8e4
F8NP = mybir.dt.np(f8)
DR = mybir.MatmulPerfMode.DoubleRow

# per-partition element offset of (sub-phase s, eighth q) block in packed x
XBLK = [KQ * w for w in SUBW]
XOFF = []
_off = 0
for _s in range(NSUB):
    XOFF.append(_off)
    _off += NQ * XBLK[_s]
XTOT = _off                  # = KC * T / P * ... per-partition elements

_CACHE = {}


def _emit_topk(nc, sc_pool, out_pool, scores, wout, iout, t0):
    """Group-limited top-k + normalize on a [128, 256] f32 logits tile."""
    scores_g = scores.rearrange("p (g e) -> p g e", g=G)
    glog = sc_pool.tile([P, G], f32, tag="glog", name="glog")
    nc.vector.reduce_max(out=glog, in_=scores_g, axis=mybir.AxisListType.X)
    gsort = sc_pool.tile([P, G], f32, tag="gsort", name="gsort")
    nc.vector.max(out=gsort, in_=glog)
    maskadd = sc_pool.tile([P, G], f32, tag="maskadd", name="maskadd")
    nc.vector.tensor_scalar(
        out=maskadd,
        in0=glog,
        scalar1=gsort[:, TOPK_GROUPS - 1:TOPK_GROUPS],
        scalar2=NEG,
        op0=mybir.AluOpType.is_lt,
        op1=mybir.AluOpType.mult,
    )
    masked = sc_pool.tile([P, E], f32, tag="masked", name="masked")
    nc.vector.tensor_add(
        masked.rearrange("p (g e) -> p g e", g=G),
        scores_g,
        maskadd.to_broadcast([P, G, EPG]),
    )
    top8 = sc_pool.tile([P, TOPK], f32, tag="top8", name="top8")
    nc.vector.max(out=top8, in_=masked)
    idx = out_pool.tile([P, TOPK], mybir.dt.uint32, tag="idx", name="idx")
    nc.vector.max_index(out=idx, in_max=top8, in_values=masked)
    sig = sc_pool.tile([P, TOPK], f32, tag="sig", name="sig")
    nc.scalar.activation(
        out=sig, in_=top8, func=mybir.ActivationFunctionType.Sigmoid
    )
    ssum = sc_pool.tile([P, 1], f32, tag="ssum", name="ssum")
    nc.vector.reduce_sum(out=ssum, in_=sig, axis=mybir.AxisListType.X)
    rec = sc_pool.tile([P, 1], f32, tag="rec", name="rec")
    nc.vector.reciprocal(out=rec, in_=ssum)
    wres = out_pool.tile([P, TOPK], f32, tag="wres", name="wres")
    nc.vector.tensor_scalar(
        out=wres,
        in0=sig,
        scalar1=rec[:, 0:1],
        scalar2=ROUTE_SCALE,
        op0=mybir.AluOpType.mult,
        op1=mybir.AluOpType.mult,
    )
    nc.gpsimd.dma_start(out=wout[t0:t0 + P, :], in_=wres)
    nc.gpsimd.dma_start(out=iout[t0:t0 + P, :], in_=idx)


def _build_v4():
    nc = bacc.Bacc("TRN2", target_bir_lowering=False, debug=False, num_devices=N_CORES)
    xh16 = nc.dram_tensor("xh16", [P, XTOT], f16, kind="ExternalInput").ap()
    xl8 = nc.dram_tensor("xl8", [P, XTOT], f8, kind="ExternalInput").ap()
    wh16 = nc.dram_tensor("wh16", [WHS, P, KWH * E], f16, kind="ExternalInput").ap()
    wh8 = nc.dram_tensor("wh8", [W8S, P, KW8 * E], f8, kind="ExternalInput").ap()
    wl8 = nc.dram_tensor("wl8", [W8S, P, KW8 * E], f8, kind="ExternalInput").ap()
    wout = nc.dram_tensor("w_out", [T, TOPK], f32, kind="ExternalOutput").ap()
    iout = nc.dram_tensor("i_out", [T, TOPK], mybir.dt.uint32, kind="ExternalOutput").ap()

    with tile.TileContext(nc) as tc, ExitStack() as ctx:
        const_pool = ctx.enter_context(tc.tile_pool(name="const", bufs=1))
        w_pool = ctx.enter_context(tc.tile_pool(name="w", bufs=1))
        xh_pool = ctx.enter_context(tc.tile_pool(name="xh", bufs=8))
        x8_pool = ctx.enter_context(tc.tile_pool(name="x8", bufs=9))
        sc_pool = ctx.enter_context(tc.tile_pool(name="scores", bufs=1))
        tk_pool = ctx.enter_context(tc.tile_pool(name="topk", bufs=2))
        out_pool = ctx.enter_context(tc.tile_pool(name="outs", bufs=4))
        ps_main = ctx.enter_context(tc.tile_pool(name="psmain", bufs=1, space="PSUM"))
        ps_res = ctx.enter_context(tc.tile_pool(name="psres", bufs=2, space="PSUM"))
        ps_tr = ctx.enter_context(tc.tile_pool(name="pstr", bufs=2, space="PSUM"))

        ident = const_pool.tile([P, P], f32, tag="ident")
        make_identity(nc, ident)

        RINGS = [nc.sync, nc.scalar]
        wh16_sb, w8_sb = [], []

        def load_w_slice(s):
            wt = w_pool.tile([P, KWH, E], f16, tag=f"wh16s{s}", name=f"wh16s{s}")
            RINGS[s % 2].dma_start(
                out=wt, in_=wh16[s].rearrange("p (k e) -> p k e", k=KWH))
            wh16_sb.append(wt)

        def load_w8_slice(s):
            w8t = w_pool.tile([P, 2, KW8, E], f8, tag=f"w8s{s}", name=f"w8s{s}")
            RINGS[(s + 1) % 2].dma_start(
                out=w8t[:, 0], in_=wh8[s].rearrange("p (k e) -> p k e", k=KW8))
            RINGS[s % 2].dma_start(
                out=w8t[:, 1], in_=wl8[s].rearrange("p (k e) -> p k e", k=KW8))
            w8_sb.append(w8t)

        def load_xh_quarter(s, q, split=False):
            w = SUBW[s]
            off = XOFF[s] + q * XBLK[s]
            src = xh16[:, off:off + KQ * w].rearrange("p (k t) -> p k t", k=KQ)
            xht = xh_pool.tile([P, KQ, w], f16, tag="xh", name=f"xh{q}_{s}")
            if split:
                h = KQ // 2 + 1
                RINGS[q % 2].dma_start(out=xht[:, :h], in_=src[:, :h])
                RINGS[q % 2].dma_start(out=xht[:, h:], in_=src[:, h:])
            else:
                RINGS[q % 2].dma_start(out=xht, in_=src)
            return xht

        def load_xl_quarter(s, q):
            w = SUBW[s]
            off = XOFF[s] + q * XBLK[s]
            x8t = x8_pool.tile([P, 2, KQ, w], f8, tag="x8", name=f"x8{q}_{s}")
            RINGS[(q + 1) % 2].dma_start(
                out=x8t[:, 0],
                in_=xl8[:, off:off + KQ * w].rearrange("p (k t) -> p k t", k=KQ))
            return x8t

        def load_x_quarter(s, q):
            return load_xh_quarter(s, q), load_xl_quarter(s, q)

        def emit_cast(s, q):
            # derive the xh8 plane on DVE (saves 1B/elem of HBM traffic)
            xht, x8t = xq[(s, q)]
            nc.vector.tensor_copy(out=x8t[:, 1], in_=xht)

        # ---- startup: deadline-ordered emission. Mains need wh slice s at
        # ~t0+3.5s us and xh eighth q at ~t0+6.1q; res (starting at main
        # k=36) needs xl8+cast and w8 slice q at ~t0+44+6.1q.
        xq = {}
        xh0 = {}

        def xl_and_cast(q):
            xq[(0, q)] = (xh0[q], load_xl_quarter(0, q))
            emit_cast(0, q)

        load_w_slice(0)
        load_w_slice(1)
        xh0[0] = load_xh_quarter(0, 0, split=True)
        xh0[1] = load_xh_quarter(0, 1)
        load_w_slice(2)
        load_w_slice(3)
        xh0[2] = load_xh_quarter(0, 2)
        load_w_slice(4)
        xh0[3] = load_xh_quarter(0, 3)
        load_w_slice(5)
        xh0[4] = load_xh_quarter(0, 4)
        load_w_slice(6)
        load_w_slice(7)
        xh0[5] = load_xh_quarter(0, 5)
        load_w_slice(8)
        load_w_slice(9)
        xl_and_cast(0)
        load_w8_slice(0)
        xl_and_cast(1)
        load_w8_slice(1)
        load_w_slice(10)
        load_w_slice(11)
        xh0[6] = load_xh_quarter(0, 6)
        xl_and_cast(2)
        load_w8_slice(2)
        load_w_slice(12)
        load_w_slice(13)
        xh0[7] = load_xh_quarter(0, 7)
        xl_and_cast(3)
        load_w8_slice(3)
        xl_and_cast(4)
        load_w8_slice(4)
        xl_and_cast(5)
        load_w8_slice(5)
        xl_and_cast(6)
        load_w8_slice(6)
        xl_and_cast(7)
        load_w8_slice(7)

        main_ps = {}
        res_ps = {}

        def emit_main(s, k):
            for eh in range(2):
                es = slice(eh * P, (eh + 1) * P)
                nc.tensor.matmul(
                    main_ps[(s, eh)],
                    wh16_sb[k // KWH][:, k % KWH, es],
                    xq[(s, k // KQ)][0][:, k % KQ, :],
                    start=(k == 0), stop=(k == KC - 1),
                )

        def emit_res(s, k):
            for eh in range(2):
                es = slice(eh * P, (eh + 1) * P)
                nc.tensor.matmul(
                    res_ps[(s, eh)],
                    w8_sb[k // KW8][:, :, k % KW8, es],
                    xq[(s, k // KQ)][1][:, :, k % KQ, :],
                    start=(k == 0), stop=(k == KC - 1),
                    perf_mode=DR,
                )

        def emit_finish_tile(s, tt):
            # transpose [e,t] -> [t,e] for one 128-token tile, then topk
            ps_t = ps_tr.tile([P, E], f32, tag="tr", name=f"tr{s}_{tt}")
            for eh in range(2):
                nc.tensor.transpose(
                    ps_t[:, eh * P:(eh + 1) * P],
                    scores_sb[(s, eh)][:, tt * P:(tt + 1) * P],
                    ident,
                )
            _emit_topk(nc, tk_pool, out_pool, ps_t, wout, iout,
                       SUBOFF[s] + tt * P)

        scores_sb = {}
        pending_finish = []

        for s in range(NSUB):
            w = SUBW[s]
            # prefetch next sub-phase's x while this one computes
            if s + 1 < NSUB:
                for q in range(NQ):
                    xq[(s + 1, q)] = load_x_quarter(s + 1, q)

            for eh in range(2):
                main_ps[(s, eh)] = ps_main.tile(
                    [P, w], f32, tag=f"m{eh}", name=f"m{eh}_{s}")
                res_ps[(s, eh)] = ps_res.tile(
                    [P, w], f32, tag=f"r{eh}", name=f"r{eh}_{s}")

            ntt = w // P
            finish_ks = {6, 18, 30, 42}

            # next sub's xh8 planes are cast on DVE spread across this sub
            def maybe_cast(k):
                if k % KQ == 0 and s + 1 < NSUB:
                    emit_cast(s + 1, k // KQ)

            if s == 0:
                LAG = 36
                for k in range(KC):
                    emit_main(s, k)
                    maybe_cast(k)
                    if k >= LAG:
                        emit_res(s, k - LAG)
                for k in range(KC - LAG, KC):
                    emit_res(s, k)
            else:
                # res runs 6 chunks ahead: covers the main-bank drain wait
                for k in range(6):
                    emit_res(s, k)
                for k in range(KC):
                    emit_main(s, k)
                    maybe_cast(k)
                    if k in finish_ks and pending_finish:
                        emit_finish_tile(*pending_finish.pop(0))
                        if ntt == 2 and pending_finish:
                            emit_finish_tile(*pending_finish.pop(0))
                    if k + 6 < KC:
                        emit_res(s, k + 6)

            # combine: scores = main + DESCALE * res  (per expert half)
            for eh in range(2):
                sc = sc_pool.tile([P, w], f32, tag=f"sc{eh}", name=f"sc{eh}_{s}")
                nc.scalar.activation(
                    out=sc, in_=res_ps[(s, eh)],
                    func=mybir.ActivationFunctionType.Copy, scale=DESCALE,
                )
                nc.vector.tensor_add(sc, sc, main_ps[(s, eh)])
                scores_sb[(s, eh)] = sc
            pending_finish += [(s, tt) for tt in range(ntt)]

        while pending_finish:
            emit_finish_tile(*pending_finish.pop(0))

    nc.compile()
    return nc


def _get_program():
    if "nc_v4" not in _CACHE:
        _CACHE["nc_v4"] = _build_v4()
    return _CACHE["nc_v4"]


def _prep_host(x, weight):
    xt = np.ascontiguousarray(x.T)                     # [D, T_FULL] f32
    xh16_full = xt.astype(np.float16)
    xl8_full = ((xt - xh16_full.astype(np.float32)) * np.float32(2.0 ** S_XL)).astype(F8NP)
    wt = np.ascontiguousarray(weight.T)                # [D, E] f32
    wh16 = wt.astype(np.float16)
    wh8 = (wh16.astype(np.float32) * np.float32(2.0 ** S_WH)).astype(F8NP)
    wl8 = ((wt - wh16.astype(np.float32)) * np.float32(2.0 ** S_WL)).astype(F8NP)

    def pack_x(a, c):
        # [D, T_FULL] -> core slice -> per-partition packed [P, XTOT]
        ac = a[:, c * T:(c + 1) * T]                   # [D, T]
        ac = ac.reshape(NQ, KQ, P, T)                  # D = NQ*KQ*P
        out = np.empty((P, XTOT), dtype=a.dtype)
        for s in range(NSUB):
            blk = ac[:, :, :, SUBOFF[s]:SUBOFF[s] + SUBW[s]]   # [NQ,KQ,P,W]
            blk = blk.transpose(2, 0, 1, 3).reshape(P, NQ * KQ * SUBW[s])
            out[:, XOFF[s]:XOFF[s] + NQ * XBLK[s]] = blk
        return out

    def pack_w(a, ns, kw):
        aw = a.reshape(ns, kw, P, E).transpose(0, 2, 1, 3)
        return np.ascontiguousarray(aw.reshape(ns, P, kw * E))

    wh16_p = pack_w(wh16, WHS, KWH)
    wh8_p = pack_w(wh8, W8S, KW8)
    wl8_p = pack_w(wl8, W8S, KW8)
    in_maps = []
    for c in range(N_CORES):
        in_maps.append({
            "xh16": pack_x(xh16_full, c),
            "xl8": pack_x(xl8_full, c),
            "wh16": wh16_p,
            "wh8": wh8_p,
            "wl8": wl8_p,
        })
    return in_maps


def kernel(x: np.ndarray, weight: np.ndarray, _trace: bool = False, **_kw):
    x = np.asarray(x, dtype=np.float32)
    weight = np.asarray(weight, dtype=np.float32)
    assert x.shape == (T_FULL, D) and weight.shape == (E, D)

    nc = _get_program()
    in_maps = _prep_host(x, weight)
    if _trace:
        import prof

        results, exec_time_ns, percore, neff_dir = prof.profiled_run(
            nc, in_maps, core_ids=list(range(N_CORES))
        )
        _CACHE["last_result"] = {
            "exec_time_ns": exec_time_ns,
            "percore": percore,
            "neff_dir": neff_dir,
        }
    else:
        res = run_bass_kernel_spmd(nc, in_maps, core_ids=list(range(N_CORES)))
        results = res.results
    w_full = np.concatenate([results[c]["w_out"] for c in range(N_CORES)], axis=0)
    i_full = np.concatenate(
        [results[c]["i_out"].astype(np.int32) for c in range(N_CORES)], axis=0
    )
    return w_full, i_full
